# revision 26
# baseline (speedup 1.0000x reference)
"""Bi-directional cross-attention kernel for Trainium2 (8 NeuronCores).

Strategy
--------
Sequence-parallel: core i owns query rows [i*512, (i+1)*512) of BOTH
attention directions and produces those rows of the final output. K/V
projections are recomputed on every core from the full h_2d/h_3d (cheaper
than an all-gather at these sizes), so no collectives are needed.

Math simplifications (done on host, exact):
  - K bias bk drops out of softmax (adds a per-query constant to every score).
  - V bias bv contributes bv @ Wo to every row (attn rows sum to 1), so it is
    folded into a single output-side bias bo_sum added before LayerNorm.
  - Q bias kept (applied during the Q^T PSUM->SBUF copy).

Device data layout (per core):
  - Everything transposed up-front on host: hT [din, n] so all matmuls are
    natural PE ops (contract over partitions).
  - Q^T, K^T stored [d_model, n] bf16.
  - V stored fp8e4m3 as [kv-chunk-pair, parity, head, 80] with a ones column
    at index 64: the fp8 DoubleRow AV matmul contracts TWO kv chunks per
    instruction and yields both O^T (rows 0-63) and the softmax denominator
    (row 64) in one accumulation.
  - Scores computed as S^T [kv, q] chunk-pairs in PSUM; the two heads of a
    pair occupy different PE row groups and stream concurrently.

Engine balance (the critical resource):
  - exp of 33.6M score elements is split between ScalarE (true ACT exp,
    fp8 out) and VectorE (Schraudolph bit-trick: p = S/ln2 + 55.67 converted
    to int8 IS the fp8e4m3 encoding of exp(S/8)).  Assignment is per-head so
    each softmax row sees one consistent approximation (bias cancels in the
    denominator).
  - PSUM->SBUF evacuations of K^T/V projections alternate Scalar/Vector.
  - softmax denominators: reciprocal_approx_fast (1 DVE op, not the 8-pass
    iterative divide), gpsimd partition_broadcast, one multiply.
  - projections are double-buffered (pj0/pj1) and spread across the whole
    attention span as deadline-paced PE filler tasks.
"""

import math

import numpy as np
import ml_dtypes

import concourse.bass as bass
import concourse.bacc as bacc
import concourse.mybir as mybir
import concourse.tile as tile
from concourse.bass_utils import run_bass_kernel_spmd

N = 4096
D2D, D3D, DM, H, DH = 256, 128, 512, 8, 64
NCORES = 8
NQ = N // NCORES          # 512 query rows per core
EPS = 1e-5
P = 128
KC = N // P               # 32 kv chunks of 128
QC = NQ // P              # 4 query chunks of 128
DMC = DM // P             # 4 d_model chunks of 128

BF = mybir.dt.bfloat16
F8 = mybir.dt.float8e4
I8 = mybir.dt.int8
F32 = mybir.dt.float32
AF = mybir.ActivationFunctionType
ALU = mybir.AluOpType

K2 = KC // 2     # 16 kv chunk-pairs (DoubleRow contracts 2 chunks per mm)
VW = 80          # padded per-head V row width in fp8 (step % 16 == 0)

# Schraudolph exp -> fp8e4m3 bit pattern: int8(S/ln2 + B) viewed as fp8
# approximates exp(S/8).  B = 8*7 - 0.33 centers the interpolation error.
EXP_A = 1.44269504
EXP_B = 55.67

# Per-direction, per-head: True -> ScalarE ACT exp, False -> VectorE trick.
EXP_ON_S = [
    [True, False, True, False, True, False, True, False],
    [True, False, True, False, True, False, True, False],
]
EVAC_S_FRAC = 0.6   # fraction of K/V PSUM evacuations done on ScalarE
PIPELINED_EMIT = True   # scores one iteration ahead of AV
OU_ON_S = True          # even-head accumulator evac on ScalarE
MUL_ON_G = True         # normalize multiplies on GpSimd

TRACE = False
_cache = {}


def _build_program():
    # Bacc (not bare Bass): its compile() splits multi-semaphore waits into
    # standalone event-semaphore instructions (TRN2 allows 1 wait per inst).
    nc = bacc.Bacc("TRN2", target_bir_lowering=False)

    # ---- I/O -----------------------------------------------------------
    # 256-contraction operands ship as fp8 so one DoubleRow matmul replaces
    # the two-matmul accumulation (the '2' axis is the contraction pair)
    hq2dT = nc.dram_tensor("hq2dT", [2, P, NQ], F8, kind="ExternalInput")
    hq3dT = nc.dram_tensor("hq3dT", [1, P, NQ], BF, kind="ExternalInput")
    hkv2dT = nc.dram_tensor("hkv2dT", [2, P, N], F8, kind="ExternalInput")
    hkv3dT = nc.dram_tensor("hkv3dT", [1, P, N], BF, kind="ExternalInput")
    Wq2d = nc.dram_tensor("Wq2d", [2, P, DM], F8, kind="ExternalInput")
    Wk3d = nc.dram_tensor("Wk3d", [1, P, DM], BF, kind="ExternalInput")
    Wv3d = nc.dram_tensor("Wv3d", [1, P, DM], BF, kind="ExternalInput")
    Wq3d = nc.dram_tensor("Wq3d", [1, P, DM], BF, kind="ExternalInput")
    Wk2d = nc.dram_tensor("Wk2d", [2, P, DM], F8, kind="ExternalInput")
    Wv2d = nc.dram_tensor("Wv2d", [2, P, DM], F8, kind="ExternalInput")
    Wo23 = nc.dram_tensor("Wo23", [4, P, DM], BF, kind="ExternalInput")
    Wo32 = nc.dram_tensor("Wo32", [4, P, DM], BF, kind="ExternalInput")
    bq2dT = nc.dram_tensor("bq2dT", [P, 4], F32, kind="ExternalInput")
    bq3dT = nc.dram_tensor("bq3dT", [P, 4], F32, kind="ExternalInput")
    bo_sum = nc.dram_tensor("bo_sum", [1, DM], F32, kind="ExternalInput")
    gamma_r = nc.dram_tensor("gamma_r", [1, DM], F32, kind="ExternalInput")
    beta_r = nc.dram_tensor("beta_r", [1, DM], F32, kind="ExternalInput")
    out = nc.dram_tensor("out", [NQ, DM], F32, kind="ExternalOutput")

    with tile.TileContext(nc) as tc:
        with (
            tc.tile_pool(name="const", bufs=1) as const,
            tc.tile_pool(name="kv", bufs=1) as kv,
            tc.tile_pool(name="epool", bufs=5) as epool,
            tc.tile_pool(name="rpool", bufs=2) as rpool,
            tc.tile_pool(name="misc", bufs=2) as misc,
            tc.tile_pool(name="psum", bufs=1, space="PSUM") as psum,
        ):
            # ---- constants / weights into SBUF -------------------------
            def load(dram, shape, dtype=BF):
                t = const.tile(shape, dtype, name=dram.name + "_sb")
                src = dram[:]
                if len(shape) == 3:
                    src = src.rearrange("o p f -> p o f")
                nc.sync.dma_start(t, src)
                return t

            # dir-0's operands first so the first projections start ASAP
            wq1 = load(Wq2d, [P, 2, DM], F8)
            hq1 = load(hq2dT, [P, 2, NQ], F8)
            hkv1 = load(hkv3dT, [P, 1, N])
            wk1 = load(Wk3d, [P, 1, DM])
            wv1 = load(Wv3d, [P, 1, DM])
            bq1 = const.tile([P, 4], F32, name="bq1")
            nc.sync.dma_start(bq1, bq2dT[:])
            wq2 = load(Wq3d, [P, 1, DM])
            hq2 = load(hq3dT, [P, 1, NQ])
            hkv2 = load(hkv2dT, [P, 2, N], F8)
            wk2 = load(Wk2d, [P, 2, DM], F8)
            wv2 = load(Wv2d, [P, 2, DM], F8)
            bq2 = const.tile([P, 4], F32, name="bq2")
            nc.sync.dma_start(bq2, bq3dT[:])
            wo1 = load(Wo23, [P, 4, DM])
            wo2 = load(Wo32, [P, 4, DM])

            bo_bc = const.tile([P, DM], F32, name="bo_bc")
            nc.sync.dma_start(bo_bc, bo_sum[:].to_broadcast([P, DM]))
            gamma_bc = const.tile([P, DM], F32, name="gamma_bc")
            nc.sync.dma_start(gamma_bc, gamma_r[:].to_broadcast([P, DM]))
            beta_bc = const.tile([P, DM], F32, name="beta_bc")
            nc.sync.dma_start(beta_bc, beta_r[:].to_broadcast([P, DM]))
            eps_sb = const.tile([P, 1], F32, name="eps_sb")
            nc.vector.memset(eps_sb, EPS)

            x1 = kv.tile([P, QC, DM], F32, name="x1")
            kt1 = kv.tile([P, DMC, N], BF, name="kt1")
            vv1 = kv.tile([P, K2, 2, H, VW], F8, name="vv1")
            qt1 = kv.tile([P, DMC, NQ], BF, name="qt1")
            ot1 = kv.tile([P, DMC, NQ], BF, name="ot1")
            kt2 = kv.tile([P, DMC, N], BF, name="kt2")
            vv2 = kv.tile([P, K2, 2, H, VW], F8, name="vv2")
            qt2 = kv.tile([P, DMC, NQ], BF, name="qt2")
            ot2 = kv.tile([P, DMC, NQ], BF, name="ot2")

            # the fp8 ones-columns used by the DoubleRow denominator rows,
            # written once (disjoint from the per-chunk V data columns)
            nc.vector.memset(vv1[:, :, :, :, DH:DH + 1], 1.0)
            nc.vector.memset(vv2[:, :, :, :, DH:DH + 1], 1.0)

            dirs = [
                dict(hq=hq1, nq=2, hkv=hkv1, nk=1, wq=wq1, wk=wk1, wv=wv1,
                     wo=wo1, bq=bq1, kt=kt1, vv=vv1, qt=qt1, ot=ot1),
                dict(hq=hq2, nq=1, hkv=hkv2, nk=2, wq=wq2, wk=wk2, wv=wv2,
                     wo=wo2, bq=bq2, kt=kt2, vv=vv2, qt=qt2, ot=ot2),
            ]

            # ---- projection emitters (double-buffered pj0/pj1 PSUM) ----
            pj_i = [0]

            def pj():
                t = psum.tile([P, 512], F32, tag=f"pj{pj_i[0] % 2}",
                              bufs=1, name="pj")
                pj_i[0] += 1
                return t

            ev_acc = [0.0]

            def evac_on_s():
                ev_acc[0] += EVAC_S_FRAC
                if ev_acc[0] >= 1.0 - 1e-9:
                    ev_acc[0] -= 1.0
                    return True
                return False

            DR = mybir.MatmulPerfMode.DoubleRow

            def emit_qt(D, c):
                ps = pj()
                if D["nq"] == 2:
                    nc.tensor.matmul(
                        ps, lhsT=D["wq"][:, :, c * P:(c + 1) * P],
                        rhs=D["hq"][:, :, :], start=True, stop=True,
                        perf_mode=DR)
                else:
                    nc.tensor.matmul(
                        ps, lhsT=D["wq"][:, 0, c * P:(c + 1) * P],
                        rhs=D["hq"][:, 0, :], start=True, stop=True)
                # bias-add rides the PSUM->SBUF copy on ScalarE
                nc.scalar.activation(
                    out=D["qt"][:, c, :], in_=ps, func=AF.Identity,
                    bias=D["bq"][:, c:c + 1])

            def emit_kt(D, c, f):
                ps = pj()
                if D["nk"] == 2:
                    nc.tensor.matmul(
                        ps, lhsT=D["wk"][:, :, c * P:(c + 1) * P],
                        rhs=D["hkv"][:, :, f * 512:(f + 1) * 512],
                        start=True, stop=True, perf_mode=DR)
                else:
                    nc.tensor.matmul(
                        ps, lhsT=D["wk"][:, 0, c * P:(c + 1) * P],
                        rhs=D["hkv"][:, 0, f * 512:(f + 1) * 512],
                        start=True, stop=True)
                dst = D["kt"][:, c, f * 512:(f + 1) * 512]
                if evac_on_s():
                    nc.scalar.activation(out=dst, in_=ps, func=AF.Identity)
                else:
                    nc.vector.tensor_copy(out=dst, in_=ps)

            def emit_v(D, k):
                ps = pj()
                if D["nk"] == 2:
                    nc.tensor.matmul(
                        ps, lhsT=D["hkv"][:, :, k * P:(k + 1) * P],
                        rhs=D["wv"][:, :, :], start=True, stop=True,
                        perf_mode=DR)
                else:
                    nc.tensor.matmul(
                        ps, lhsT=D["hkv"][:, 0, k * P:(k + 1) * P],
                        rhs=D["wv"][:, 0, :], start=True, stop=True)
                dst = D["vv"][:, k // 2, k % 2, :, 0:DH]
                src = ps.rearrange("p (h e) -> p h e", h=H)
                if evac_on_s():
                    nc.scalar.activation(out=dst, in_=src, func=AF.Identity)
                else:
                    nc.vector.tensor_copy(out=dst, in_=src)

            def emit_outproj(D, q, d):
                ps = pj()
                for mc in range(DMC):
                    nc.tensor.matmul(
                        ps, lhsT=D["ot"][:, mc, q * P:(q + 1) * P],
                        rhs=D["wo"][:, mc, :],
                        start=(mc == 0), stop=(mc == DMC - 1))
                if d == 0:
                    # x1 = o23 + bo_sum
                    nc.vector.tensor_add(out=x1[:, q, :], in0=ps, in1=bo_bc)
                else:
                    # x = o32 + x1 ; LayerNorm ; write out
                    x_t = misc.tile([P, DM], F32, tag="x_t", name="x_t")
                    nc.vector.tensor_add(out=x_t, in0=ps, in1=x1[:, q, :])
                    stats = misc.tile([P, 6], F32, tag="stats", name="stats")
                    nc.vector.bn_stats(out=stats, in_=x_t)
                    mv = misc.tile([P, 2], F32, tag="mv", name="mv")
                    nc.vector.bn_aggr(out=mv, in_=stats)
                    std = misc.tile([P, 1], F32, tag="std", name="std")
                    nc.scalar.activation(out=std, in_=mv[:, 1:2],
                                         func=AF.Sqrt, bias=eps_sb[:, 0:1])
                    rstd = misc.tile([P, 1], F32, tag="rstd", name="rstd")
                    nc.vector.reciprocal(rstd, std)
                    nc.vector.tensor_scalar(
                        out=x_t, in0=x_t, scalar1=mv[:, 0:1],
                        scalar2=rstd, op0=ALU.subtract, op1=ALU.mult)
                    nc.vector.tensor_mul(out=x_t, in0=x_t, in1=gamma_bc)
                    nc.vector.tensor_add(out=x_t, in0=x_t, in1=beta_bc)
                    nc.sync.dma_start(out[q * P:(q + 1) * P, :], x_t)

            # ---- deadline-paced filler tasks ---------------------------
            # global attention iterations: dir-0 -> 0..63, dir-1 -> 64..127
            D0, D1 = dirs
            tasks = []
            for k in range(4, KC):
                tasks.append((max(0, k // 2 - 1), lambda k=k: emit_v(D0, k)))
            for c in range(1, DMC):
                for f in range(N // 512):
                    tasks.append((16 * c - 2,
                                  lambda c=c, f=f: emit_kt(D0, c, f)))
            for c in range(DMC):
                tasks.append((60 + 16 * c, lambda c=c: emit_qt(D1, c)))
            for c in range(DMC):
                for f in range(N // 512):
                    tasks.append((62 + 16 * c,
                                  lambda c=c, f=f: emit_kt(D1, c, f)))
            for k in range(KC):
                tasks.append((62 + k // 2, lambda k=k: emit_v(D1, k)))
            for q in range(QC):
                tasks.append((76 + 14 * q,
                              lambda q=q: emit_outproj(D0, q, 0)))
            tasks.sort(key=lambda x: x[0])
            n_tasks = len(tasks)
            fstate = {"done": 0, "it": 0}

            def filler():
                t = fstate["it"]
                fstate["it"] += 1
                uniform = math.ceil(n_tasks * (t + 1) / 122.0)
                while fstate["done"] < n_tasks and (
                        tasks[fstate["done"]][0] <= t
                        or fstate["done"] < uniform):
                    tasks[fstate["done"]][1]()
                    fstate["done"] += 1

            def drain_tasks():
                while fstate["done"] < n_tasks:
                    tasks[fstate["done"]][1]()
                    fstate["done"] += 1

            # ---- attention ---------------------------------------------
            deferred = []
            half_state = {}

            def pop_deferred():
                if deferred:
                    deferred.pop(0)()

            def flush_deferred():
                while deferred:
                    deferred.pop(0)()

            def attention(d, D):
                kt, vv, qt, ot = D["kt"], D["vv"], D["qt"], D["ot"]

                def emit_scores(pair, k2):
                    heads = (2 * pair, 2 * pair + 1)
                    s = {}
                    for h in heads:
                        s[h] = psum.tile([P, 1024], F32, tag=f"s{h % 2}",
                                         name=f"s_ps{h % 2}")
                    # interleave the two heads: adjacent matmuls sit in
                    # different PE row groups and stream concurrently
                    for j in range(2):
                        k = 2 * k2 + j
                        for h in heads:
                            c, po = h // 2, (h % 2) * DH
                            nc.tensor.matmul(
                                s[h][:, j * 512:(j + 1) * 512],
                                lhsT=kt[po:po + DH, c, k * P:(k + 1) * P],
                                rhs=qt[po:po + DH, c, :],
                                start=True, stop=True)
                    return s

                def emit_exp(pair, k2, s):
                    e = {}
                    for h in (2 * pair, 2 * pair + 1):
                        e[h] = epool.tile([P, 2, 512], F8, tag="E",
                                          name="e_t")
                        src = s[h][:].rearrange("p (a b) -> p a b", a=2)
                        if EXP_ON_S[d][h]:
                            nc.scalar.activation(out=e[h], in_=src,
                                                 func=AF.Exp, scale=0.125)
                        else:
                            nc.vector.tensor_scalar(
                                out=e[h][:].bitcast(I8), in0=src,
                                scalar1=EXP_A, scalar2=EXP_B,
                                op0=ALU.mult, op1=ALU.add)
                    return e

                for pair in range(H // 2):
                    heads = (2 * pair, 2 * pair + 1)
                    o_ps = {h: psum.tile([P, 512], F32, tag=f"o{h % 2}",
                                         name=f"o_ps{h % 2}")
                            for h in heads}
                    def emit_av(k2, e):
                        for h in heads:
                            # fp8 DoubleRow AV: contracts both kv chunks in
                            # one matmul; vv col 64 is ones so row 64
                            # accumulates the softmax denominator
                            nc.tensor.matmul(
                                o_ps[h][0:DH + 1, :],
                                lhsT=vv[:, k2, :, h, 0:DH + 1],
                                rhs=e[h],
                                start=(k2 == 0), stop=(k2 == K2 - 1),
                                perf_mode=mybir.MatmulPerfMode.DoubleRow,
                                skip_group_check=True)

                    if PIPELINED_EMIT:
                        e_cur = emit_exp(pair, 0, emit_scores(pair, 0))
                        for k2 in range(K2):
                            if k2 + 1 < K2:
                                e_nxt = emit_exp(pair, k2 + 1,
                                                 emit_scores(pair, k2 + 1))
                            # previous pairs' normalization work, one piece
                            # per iteration (their o banks were already
                            # freed by the ou evacuations)
                            pop_deferred()
                            filler()
                            emit_av(k2, e_cur)
                            if k2 + 1 < K2:
                                e_cur = e_nxt
                    else:
                        for k2 in range(K2):
                            e = emit_exp(pair, k2, emit_scores(pair, k2))
                            pop_deferred()
                            emit_av(k2, e)
                            filler()
                    # pair end: evacuate both accumulators fast (frees the
                    # o banks before the next pair's first AV) ...
                    if pair % 2 == 0:
                        half_state["den"] = rpool.tile(
                            [4, 512], F32, tag="den", name="den", bufs=1)
                        half_state["ou"] = []
                    den = half_state["den"]
                    for h in heads:
                        t = rpool.tile([DH + 1, 512], F32, tag="ou",
                                       name="ou", bufs=4)
                        if h % 2 == 0 and OU_ON_S:
                            nc.scalar.activation(out=t,
                                                 in_=o_ps[h][0:DH + 1, :],
                                                 func=AF.Identity)
                        else:
                            nc.vector.tensor_copy(out=t,
                                                  in_=o_ps[h][0:DH + 1, :])
                        # gather this head's softmax denominators as row
                        # h%4 of den, so one reciprocal serves 4 heads
                        nc.sync.dma_start(den[h % 4:h % 4 + 1, :],
                                          t[DH:DH + 1, :])
                        half_state["ou"].append((h, t))
                    if pair % 2 == 1:
                        # normalize the 4 finished heads lazily, one piece
                        # per upcoming iteration (eagerly for the very last
                        # half so the output projection can start sooner)
                        ous = half_state["ou"]

                        def recip(den=den):
                            rden = rpool.tile([4, 512], F32, tag="rden",
                                              name="rden", bufs=1)
                            half_state["rden"] = rden
                            nc.vector.reciprocal(rden, den)
                        items = [recip]
                        for h, t in ous:
                            items.append(
                                lambda h=h, t=t: _normalize(
                                    ot, h, t,
                                    half_state["rden"][h % 4:h % 4 + 1, :]))
                        if d == 1 and pair == H // 2 - 1:
                            for it in items:
                                it()
                        else:
                            deferred.extend(items)

            def _normalize(ot, h, ou_t, rden_row):
                c, po = h // 2, (h % 2) * DH
                # partition_broadcast needs its source at partition 0
                rc = rpool.tile([1, 512], F32, tag="rcp", name="rcp",
                                bufs=1)
                nc.sync.dma_start(rc[:], rden_row)
                rb = rpool.tile([DH, 512], F32, tag="rbc", name="rbc")
                nc.gpsimd.partition_broadcast(rb[:], rc[:])
                mul = (nc.gpsimd.tensor_tensor if MUL_ON_G
                       else nc.vector.tensor_tensor)
                if po == 0:
                    mul(out=ot[0:DH, c, :], in0=ou_t[0:DH, :], in1=rb,
                        op=ALU.mult)
                else:
                    # engines cannot shift partitions: stage at base 0,
                    # then DMA into partitions 64-127.
                    st = rpool.tile([DH, 512], BF, tag="stg", name="stg")
                    mul(out=st, in0=ou_t[0:DH, :], in1=rb, op=ALU.mult)
                    nc.sync.dma_start(ot[po:po + DH, c, :], st)

            # ---- emission schedule -------------------------------------
            # phase A: just enough of dir-0's projections to start pair 0
            for c in range(DMC):
                emit_qt(D0, c)
            for f in range(N // 512):
                emit_kt(D0, 0, f)
            for k in range(4):
                emit_v(D0, k)

            attention(0, D0)
            attention(1, D1)
            drain_tasks()
            flush_deferred()

            # dir-1 output projection + residual + LayerNorm + store
            for q in range(QC):
                emit_outproj(D1, q, 1)

    nc.compile()
    return nc


def _prep_inputs(inputs):
    bf = ml_dtypes.bfloat16
    f = {k: np.asarray(v, dtype=np.float32) for k, v in inputs.items()}

    f8 = ml_dtypes.float8_e4m3fn
    h2dT = np.ascontiguousarray(f["h_2d"].T).astype(f8)      # [256, 4096]
    h3dT = np.ascontiguousarray(f["h_3d"].T).astype(bf)      # [128, 4096]

    def wchunk(w, dt=bf):
        w = np.asarray(w, dtype=np.float32).astype(dt)
        return np.ascontiguousarray(w.reshape(-1, P, DM))

    bo = (f["bo23"].astype(np.float64)
          + f["bv3d"].astype(np.float64) @ f["Wo23"].astype(np.float64)
          + f["bo32"].astype(np.float64)
          + f["bv2d"].astype(np.float64) @ f["Wo32"].astype(np.float64))

    common = {
        "hkv2dT": np.ascontiguousarray(h2dT.reshape(2, P, N)),
        "hkv3dT": np.ascontiguousarray(h3dT.reshape(1, P, N)),
        "Wq2d": wchunk(f["Wq2d"], f8), "Wk3d": wchunk(f["Wk3d"]),
        "Wv3d": wchunk(f["Wv3d"]), "Wq3d": wchunk(f["Wq3d"]),
        "Wk2d": wchunk(f["Wk2d"], f8), "Wv2d": wchunk(f["Wv2d"], f8),
        "Wo23": wchunk(f["Wo23"]), "Wo32": wchunk(f["Wo32"]),
        "bq2dT": np.ascontiguousarray(f["bq2d"].reshape(4, P).T),
        "bq3dT": np.ascontiguousarray(f["bq3d"].reshape(4, P).T),
        "bo_sum": np.ascontiguousarray(bo.astype(np.float32)[None, :]),
        "gamma_r": np.ascontiguousarray(f["gamma"][None, :]),
        "beta_r": np.ascontiguousarray(f["beta"][None, :]),
    }

    in_maps = []
    for i in range(NCORES):
        sl = slice(i * NQ, (i + 1) * NQ)
        m = dict(common)
        m["hq2dT"] = np.ascontiguousarray(h2dT[:, sl]).reshape(2, P, NQ)
        m["hq3dT"] = np.ascontiguousarray(h3dT[:, sl]).reshape(1, P, NQ)
        in_maps.append(m)
    return in_maps


def kernel(**inputs) -> np.ndarray:
    if "nc" not in _cache:
        _cache["nc"] = _build_program()
    nc = _cache["nc"]
    in_maps = _prep_inputs(inputs)
    res = run_bass_kernel_spmd(nc, in_maps, core_ids=list(range(NCORES)),
                               trace=TRACE)
    _cache["last_result"] = res
    return np.concatenate([r["out"] for r in res.results], axis=0)


# revision 64
# speedup vs baseline: 1.8568x; 1.8568x over previous
"""Bi-directional cross-attention kernel for Trainium2 (8 NeuronCores).

Strategy
--------
Sequence-parallel: core i owns query rows [i*512, (i+1)*512) of BOTH
attention directions and produces those rows of the final output. K/V
projections are recomputed on every core from the full h_2d/h_3d (cheaper
than an all-gather at these sizes), so no collectives are needed.

Math simplifications (done on host, exact):
  - K bias bk drops out of softmax (adds a per-query constant to every score).
  - V bias bv contributes bv @ Wo to every row (attn rows sum to 1), so it is
    folded into a single output-side bias bo_sum added before LayerNorm.
  - Q bias kept (applied during the Q^T PSUM->SBUF copy).

Device data layout (per core):
  - Everything transposed up-front on host: hT [din, n] so all matmuls are
    natural PE ops (contract over partitions).
  - Q^T, K^T stored [d_model, n] bf16.
  - V stored fp8e4m3 as [kv-chunk-pair, parity, head, 80] with a ones column
    at index 64: the fp8 DoubleRow AV matmul contracts TWO kv chunks per
    instruction and yields both O^T (rows 0-63) and the softmax denominator
    (row 64) in one accumulation.
  - Scores computed as S^T [kv, q] chunk-pairs in PSUM; the two heads of a
    pair occupy different PE row groups and stream concurrently.

Engine balance (the critical resource):
  - exp of 33.6M score elements is split between ScalarE (true ACT exp,
    fp8 out) and VectorE (Schraudolph bit-trick: p = S/ln2 + 55.67 converted
    to int8 IS the fp8e4m3 encoding of exp(S/8)).  Assignment is per-head so
    each softmax row sees one consistent approximation (bias cancels in the
    denominator).
  - scores live in four single-bank [P,512] PSUM tiles with one exp per
    tile: the scores->exp->bank-free round trip is half an iteration, so
    the PE never idles long enough for the HAM clock-gate to throttle the
    array to 1.2 GHz (the dominant cost of earlier versions).
  - PSUM->SBUF evacuations of K^T/V projections alternate Scalar/Vector.
  - softmax denominators: each head's row is parked in DRAM, 4 heads come
    back as one partition-major [128,16] tile (reciprocal costs ~16
    elements/lane instead of 512), and 1/den returns via broadcast-DMAs.
  - projections are double-buffered (pj0/pj1) and spread across the whole
    attention span as deadline-paced PE filler tasks; the dir-1 output
    projection's first chunks run during the last attention pair.
"""

import math

import numpy as np
import ml_dtypes

import concourse.bass as bass
import concourse.bacc as bacc
import concourse.mybir as mybir
import concourse.tile as tile
from concourse.bass_utils import run_bass_kernel_spmd

N = 4096
D2D, D3D, DM, H, DH = 256, 128, 512, 8, 64
NCORES = 8
NQ = N // NCORES          # 512 query rows per core
EPS = 1e-5
P = 128
KC = N // P               # 32 kv chunks of 128
QC = NQ // P              # 4 query chunks of 128
DMC = DM // P             # 4 d_model chunks of 128

BF = mybir.dt.bfloat16
F8 = mybir.dt.float8e4
I8 = mybir.dt.int8
F32 = mybir.dt.float32
AF = mybir.ActivationFunctionType
ALU = mybir.AluOpType

K2 = KC // 2     # 16 kv chunk-pairs (DoubleRow contracts 2 chunks per mm)
VW = 80          # padded per-head V row width in fp8 (step % 16 == 0)

# Schraudolph exp -> fp8e4m3 bit pattern: int8(S/ln2 + B) viewed as fp8
# approximates exp(S/8).  B = 8*7 - 0.33 centers the interpolation error.
EXP_A = 1.44269504
EXP_B = 55.67

# Per-direction, per-head: True -> ScalarE ACT exp, False -> VectorE trick.
EXP_ON_S = [
    [True, False, True, False, True, False, True, False],
    [True, False, True, False, True, False, True, False],
]
EVAC_S_FRAC = 0.58  # fraction of K/V PSUM evacuations done on ScalarE
PIPELINED_EMIT = True   # scores one iteration ahead of AV
OU_ON_S = True          # even-head accumulator evac on ScalarE
MUL_ON_G = False        # gpsimd tensor_tensor thrashes IRAM libs: keep off
HEAT_START = 10         # warmup heater matmuls before phase A
HEAT_PAIR = 0           # heater matmuls at each attention pair start

TRACE = False
_cache = {}


def _build_program(apply_gb=True):
    # Bacc (not bare Bass): its compile() splits multi-semaphore waits into
    # standalone event-semaphore instructions (TRN2 allows 1 wait per inst).
    nc = bacc.Bacc("TRN2", target_bir_lowering=False)

    # ---- I/O -----------------------------------------------------------
    hq2dT = nc.dram_tensor("hq2dT", [2, P, NQ], BF, kind="ExternalInput")
    hq3dT = nc.dram_tensor("hq3dT", [1, P, NQ], BF, kind="ExternalInput")
    hkv2dT = nc.dram_tensor("hkv2dT", [2, P, N], BF, kind="ExternalInput")
    hkv3dT = nc.dram_tensor("hkv3dT", [1, P, N], BF, kind="ExternalInput")
    Wq2d = nc.dram_tensor("Wq2d", [2, P, DM], BF, kind="ExternalInput")
    Wk3d = nc.dram_tensor("Wk3d", [1, P, DM], BF, kind="ExternalInput")
    Wv3d = nc.dram_tensor("Wv3d", [1, P, DM], BF, kind="ExternalInput")
    Wq3d = nc.dram_tensor("Wq3d", [1, P, DM], BF, kind="ExternalInput")
    Wk2d = nc.dram_tensor("Wk2d", [2, P, DM], BF, kind="ExternalInput")
    Wv2d = nc.dram_tensor("Wv2d", [2, P, DM], BF, kind="ExternalInput")
    Wo23 = nc.dram_tensor("Wo23", [4, P, DM], BF, kind="ExternalInput")
    Wo32 = nc.dram_tensor("Wo32", [4, P, DM], BF, kind="ExternalInput")
    bq2dT = nc.dram_tensor("bq2dT", [P, 4], F32, kind="ExternalInput")
    bq3dT = nc.dram_tensor("bq3dT", [P, 4], F32, kind="ExternalInput")
    bo_sum = nc.dram_tensor("bo_sum", [1, DM], F32, kind="ExternalInput")
    gamma_r = nc.dram_tensor("gamma_r", [1, DM], F32, kind="ExternalInput")
    beta_r = nc.dram_tensor("beta_r", [1, DM], F32, kind="ExternalInput")
    out = nc.dram_tensor("out", [NQ, DM], F32, kind="ExternalOutput")
    # DRAM bounce buffers for the softmax denominators: SBUF-source DMAs
    # cannot broadcast or cross partitions, DRAM-source ones can.  den_dr
    # collects 4 heads' denominator rows; they come back as a [128,16]
    # partition-major tile so one reciprocal costs ~16 elements/lane.
    den_dr = nc.dram_tensor("den_dr", [4, 4, 1, 512], F32, kind="Internal")
    rden_dr = nc.dram_tensor("rden_dr", [4, 4, 1, 512], F32, kind="Internal")

    with tile.TileContext(nc) as tc:
        with (
            tc.tile_pool(name="const", bufs=1) as const,
            tc.tile_pool(name="kv", bufs=1) as kv,
            tc.tile_pool(name="epool", bufs=5) as epool,
            tc.tile_pool(name="rpool", bufs=2) as rpool,
            tc.tile_pool(name="misc", bufs=2) as misc,
            tc.tile_pool(name="psum", bufs=1, space="PSUM") as psum,
        ):
            # ---- constants / weights into SBUF -------------------------
            def load(dram, shape, dtype=BF):
                t = const.tile(shape, dtype, name=dram.name + "_sb")
                src = dram[:]
                if len(shape) == 3:
                    src = src.rearrange("o p f -> p o f")
                nc.sync.dma_start(t, src)
                return t

            # dir-0's operands first so the first projections start ASAP
            wq1 = load(Wq2d, [P, 2, DM])
            hq1 = load(hq2dT, [P, 2, NQ])
            hkv1 = load(hkv3dT, [P, 1, N])
            wk1 = load(Wk3d, [P, 1, DM])
            wv1 = load(Wv3d, [P, 1, DM])
            bq1 = const.tile([P, 4], F32, name="bq1")
            nc.sync.dma_start(bq1, bq2dT[:])
            wq2 = load(Wq3d, [P, 1, DM])
            hq2 = load(hq3dT, [P, 1, NQ])
            hkv2 = load(hkv2dT, [P, 2, N])
            wk2 = load(Wk2d, [P, 2, DM])
            wv2 = load(Wv2d, [P, 2, DM])
            bq2 = const.tile([P, 4], F32, name="bq2")
            nc.sync.dma_start(bq2, bq3dT[:])
            wo1 = load(Wo23, [P, 4, DM])
            wo2 = load(Wo32, [P, 4, DM])

            bo_bc = const.tile([P, DM], F32, name="bo_bc")
            nc.sync.dma_start(bo_bc, bo_sum[:].to_broadcast([P, DM]))
            gamma_bc = const.tile([P, DM], F32, name="gamma_bc")
            nc.sync.dma_start(gamma_bc, gamma_r[:].to_broadcast([P, DM]))
            beta_bc = const.tile([P, DM], F32, name="beta_bc")
            nc.sync.dma_start(beta_bc, beta_r[:].to_broadcast([P, DM]))
            eps_sb = const.tile([P, 1], F32, name="eps_sb")
            nc.vector.memset(eps_sb, EPS)

            # HAM heater: dummy matmuls keep the PE activity monitor from
            # throttling the array clock to 1.2 GHz during engine-bound
            # stretches. Reads a memset tile, writes an unread pj tile.
            heat = const.tile([P, 256], BF, name="heat")
            nc.vector.memset(heat, 0.0)

            def heater(n):
                for _ in range(n):
                    nc.tensor.matmul(pj()[:, 0:256], lhsT=heat[:, 0:P],
                                     rhs=heat[:], start=True, stop=True)

            x1 = kv.tile([P, QC, DM], F32, name="x1")
            kt1 = kv.tile([P, DMC, N], BF, name="kt1")
            vv1 = kv.tile([P, K2, 2, H, VW], F8, name="vv1")
            qt1 = kv.tile([P, DMC, NQ], BF, name="qt1")
            ot1 = kv.tile([P, DMC, NQ], BF, name="ot1")
            kt2 = kv.tile([P, DMC, N], BF, name="kt2")
            vv2 = kv.tile([P, K2, 2, H, VW], F8, name="vv2")
            qt2 = kv.tile([P, DMC, NQ], BF, name="qt2")
            ot2 = kv.tile([P, DMC, NQ], BF, name="ot2")

            # the fp8 ones-columns used by the DoubleRow denominator rows,
            # written once (disjoint from the per-chunk V data columns)
            nc.vector.memset(vv1[:, :, :, :, DH:DH + 1], 1.0)
            nc.vector.memset(vv2[:, :, :, :, DH:DH + 1], 1.0)

            dirs = [
                dict(hq=hq1, nq=2, hkv=hkv1, nk=1, wq=wq1, wk=wk1, wv=wv1,
                     wo=wo1, bq=bq1, kt=kt1, vv=vv1, qt=qt1, ot=ot1),
                dict(hq=hq2, nq=1, hkv=hkv2, nk=2, wq=wq2, wk=wk2, wv=wv2,
                     wo=wo2, bq=bq2, kt=kt2, vv=vv2, qt=qt2, ot=ot2),
            ]

            # ---- projection emitters (double-buffered pj0/pj1 PSUM) ----
            pj_i = [0]

            def pj():
                t = psum.tile([P, 512], F32, tag=f"pj{pj_i[0] % 2}",
                              bufs=1, name="pj")
                pj_i[0] += 1
                return t

            ev_acc = [0.0]

            def evac_on_s():
                ev_acc[0] += EVAC_S_FRAC
                if ev_acc[0] >= 1.0 - 1e-9:
                    ev_acc[0] -= 1.0
                    return True
                return False

            def emit_qt(D, c):
                ps = pj()
                for dc in range(D["nq"]):
                    nc.tensor.matmul(
                        ps, lhsT=D["wq"][:, dc, c * P:(c + 1) * P],
                        rhs=D["hq"][:, dc, :],
                        start=(dc == 0), stop=(dc == D["nq"] - 1))
                # bias-add rides the PSUM->SBUF copy on ScalarE
                nc.scalar.activation(
                    out=D["qt"][:, c, :], in_=ps, func=AF.Identity,
                    bias=D["bq"][:, c:c + 1])

            def emit_kt(D, c, f):
                ps = pj()
                for dc in range(D["nk"]):
                    nc.tensor.matmul(
                        ps, lhsT=D["wk"][:, dc, c * P:(c + 1) * P],
                        rhs=D["hkv"][:, dc, f * 512:(f + 1) * 512],
                        start=(dc == 0), stop=(dc == D["nk"] - 1))
                dst = D["kt"][:, c, f * 512:(f + 1) * 512]
                if evac_on_s():
                    nc.scalar.activation(out=dst, in_=ps, func=AF.Identity)
                else:
                    nc.vector.tensor_copy(out=dst, in_=ps)

            def emit_v(D, k):
                ps = pj()
                for dc in range(D["nk"]):
                    nc.tensor.matmul(
                        ps, lhsT=D["hkv"][:, dc, k * P:(k + 1) * P],
                        rhs=D["wv"][:, dc, :],
                        start=(dc == 0), stop=(dc == D["nk"] - 1))
                dst = D["vv"][:, k // 2, k % 2, :, 0:DH]
                src = ps.rearrange("p (h e) -> p h e", h=H)
                if evac_on_s():
                    nc.scalar.activation(out=dst, in_=src, func=AF.Identity)
                else:
                    nc.vector.tensor_copy(out=dst, in_=src)

            def outproj_mms(D, q, ps, mc0, mc1, start):
                for mc in range(mc0, mc1):
                    nc.tensor.matmul(
                        ps, lhsT=D["ot"][:, mc, q * P:(q + 1) * P],
                        rhs=D["wo"][:, mc, :],
                        start=(start and mc == mc0),
                        stop=(mc == DMC - 1))

            def emit_outproj(D, q, d):
                ps = pj()
                outproj_mms(D, q, ps, 0, DMC, True)
                finish_outproj(ps, q, d)

            # outproj1 rows q=0,1 run their first 3 contraction chunks
            # during dir-1's last pair (inputs already normalized), so the
            # tail only owes them one matmul + LayerNorm each
            op1_ps = {}

            def op1_partial(q):
                ps = pj()
                op1_ps[q] = ps
                outproj_mms(dirs[1], q, ps, 0, 2, True)

            def op1_final(q):
                if q in op1_ps:
                    ps = op1_ps[q]
                    outproj_mms(dirs[1], q, ps, 2, DMC, False)
                else:
                    ps = pj()
                    outproj_mms(dirs[1], q, ps, 0, DMC, True)
                finish_outproj(ps, q, 1)

            def finish_outproj(ps, q, d):
                if d == 0:
                    # x1 = o23 + bo_sum
                    nc.vector.tensor_add(out=x1[:, q, :], in0=ps, in1=bo_bc)
                else:
                    # x = o32 + x1 ; LayerNorm ; write out
                    x_t = misc.tile([P, DM], F32, tag="x_t", name="x_t")
                    nc.vector.tensor_add(out=x_t, in0=ps, in1=x1[:, q, :])
                    stats = misc.tile([P, 6], F32, tag="stats", name="stats")
                    nc.vector.bn_stats(out=stats, in_=x_t)
                    mv = misc.tile([P, 2], F32, tag="mv", name="mv")
                    nc.vector.bn_aggr(out=mv, in_=stats)
                    std = misc.tile([P, 1], F32, tag="std", name="std")
                    nc.scalar.activation(out=std, in_=mv[:, 1:2],
                                         func=AF.Sqrt, bias=eps_sb[:, 0:1])
                    rstd = misc.tile([P, 1], F32, tag="rstd", name="rstd")
                    nc.vector.reciprocal(rstd, std)
                    nc.vector.tensor_scalar(
                        out=x_t, in0=x_t, scalar1=mv[:, 0:1],
                        scalar2=rstd, op0=ALU.subtract, op1=ALU.mult)
                    if apply_gb:
                        nc.vector.tensor_mul(out=x_t, in0=x_t,
                                             in1=gamma_bc)
                        nc.vector.tensor_add(out=x_t, in0=x_t,
                                             in1=beta_bc)
                    nc.sync.dma_start(out[q * P:(q + 1) * P, :], x_t)

            # ---- deadline-paced filler tasks ---------------------------
            # global attention iterations: dir-0 -> 0..63, dir-1 -> 64..127
            D0, D1 = dirs
            tasks = []
            for k in range(4, KC):
                tasks.append((max(0, k // 2 - 1), lambda k=k: emit_v(D0, k)))
            for c in range(1, DMC):
                for f in range(N // 512):
                    tasks.append((16 * c - 2,
                                  lambda c=c, f=f: emit_kt(D0, c, f)))
            for c in range(DMC):
                tasks.append((60 + 16 * c, lambda c=c: emit_qt(D1, c)))
            for c in range(DMC):
                for f in range(N // 512):
                    tasks.append((62 + 16 * c,
                                  lambda c=c, f=f: emit_kt(D1, c, f)))
            for k in range(KC):
                tasks.append((62 + k // 2, lambda k=k: emit_v(D1, k)))
            for q in range(QC):
                tasks.append((74 + 12 * q,
                              lambda q=q: emit_outproj(D0, q, 0)))
            tasks.sort(key=lambda x: x[0])
            n_tasks = len(tasks)
            fstate = {"done": 0, "it": 0}

            def filler():
                t = fstate["it"]
                fstate["it"] += 1
                uniform = math.ceil(n_tasks * (t + 1) / 122.0)
                while fstate["done"] < n_tasks and (
                        tasks[fstate["done"]][0] <= t
                        or fstate["done"] < uniform):
                    tasks[fstate["done"]][1]()
                    fstate["done"] += 1

            def drain_tasks():
                while fstate["done"] < n_tasks:
                    tasks[fstate["done"]][1]()
                    fstate["done"] += 1

            # ---- attention ---------------------------------------------
            deferred = []
            half_state = {}

            def pop_deferred():
                if deferred:
                    deferred.pop(0)()

            def flush_deferred():
                while deferred:
                    deferred.pop(0)()

            def attention(d, D):
                kt, vv, qt, ot = D["kt"], D["vv"], D["qt"], D["ot"]
                srot = [0]

                def emit_scores_exp(pair, k2):
                    # [P,512] score tiles + per-half exp: the bank a score
                    # matmul writes is freed by one 512-wide exp, halving
                    # the scores->exp->reuse round trip that stalls the PE
                    heads = (2 * pair, 2 * pair + 1)
                    e = {}
                    for h in heads:
                        e[h] = epool.tile([P, 2, 512], F8, tag="E",
                                          name="e_t")
                    for j in range(2):
                        k = 2 * k2 + j
                        s = {}
                        for h in heads:
                            s[h] = psum.tile([P, 512], F32,
                                             tag=f"u{srot[0] % 4}",
                                             name="s_u")
                            srot[0] += 1
                        # the two heads sit in different PE row groups and
                        # stream concurrently
                        for h in heads:
                            c, po = h // 2, (h % 2) * DH
                            nc.tensor.matmul(
                                s[h],
                                lhsT=kt[po:po + DH, c, k * P:(k + 1) * P],
                                rhs=qt[po:po + DH, c, :],
                                start=True, stop=True)
                        for h in heads:
                            dst = e[h][:, j, :]
                            if EXP_ON_S[d][h]:
                                nc.scalar.activation(out=dst, in_=s[h],
                                                     func=AF.Exp,
                                                     scale=0.125)
                            else:
                                nc.vector.tensor_scalar(
                                    out=dst.bitcast(I8), in0=s[h],
                                    scalar1=EXP_A, scalar2=EXP_B,
                                    op0=ALU.mult, op1=ALU.add)
                    return e

                for pair in range(H // 2):
                    if d == 1 and pair == H // 2 - 1:
                        deferred.append(lambda: op1_partial(0))
                        deferred.append(lambda: op1_partial(1))
                    heads = (2 * pair, 2 * pair + 1)
                    o_ps = {h: psum.tile([P, 512], F32, tag=f"o{h % 2}",
                                         name=f"o_ps{h % 2}")
                            for h in heads}
                    def emit_av(k2, e):
                        for h in heads:
                            # fp8 DoubleRow AV: contracts both kv chunks in
                            # one matmul; vv col 64 is ones so row 64
                            # accumulates the softmax denominator
                            nc.tensor.matmul(
                                o_ps[h][0:DH + 1, :],
                                lhsT=vv[:, k2, :, h, 0:DH + 1],
                                rhs=e[h],
                                start=(k2 == 0), stop=(k2 == K2 - 1),
                                perf_mode=mybir.MatmulPerfMode.DoubleRow,
                                skip_group_check=True)

                    if PIPELINED_EMIT:
                        e_cur = emit_scores_exp(pair, 0)
                        if HEAT_PAIR:
                            heater(HEAT_PAIR)
                        for k2 in range(K2):
                            if k2 + 1 < K2:
                                e_nxt = emit_scores_exp(pair, k2 + 1)
                            # previous pairs' normalization work, one piece
                            # per iteration (their o banks were already
                            # freed by the ou evacuations)
                            pop_deferred()
                            filler()
                            emit_av(k2, e_cur)
                            if k2 + 1 < K2:
                                e_cur = e_nxt
                    else:
                        for k2 in range(K2):
                            e = emit_scores_exp(pair, k2)
                            pop_deferred()
                            emit_av(k2, e)
                            filler()
                    # pair end: evacuate both accumulators fast (frees the
                    # o banks before the next pair's first AV) ...
                    if pair % 2 == 0:
                        half_state["ou"] = []
                    half_ix = d * 2 + pair // 2
                    for h in heads:
                        t = rpool.tile([DH + 1, 512], F32, tag="ou",
                                       name="ou", bufs=4)
                        if h % 2 == 0 and OU_ON_S:
                            nc.scalar.activation(out=t,
                                                 in_=o_ps[h][0:DH + 1, :],
                                                 func=AF.Identity)
                        else:
                            nc.vector.tensor_copy(out=t,
                                                  in_=o_ps[h][0:DH + 1, :])
                        # park this head's denominator row in DRAM; one
                        # reciprocal serves 4 heads
                        nc.sync.dma_start(den_dr[half_ix, h % 4],
                                          t[DH:DH + 1, :])
                        half_state["ou"].append((h, t))
                    if pair % 2 == 1:
                        # normalize the 4 finished heads lazily, one piece
                        # per upcoming iteration (eagerly for the very last
                        # half so the output projection can start sooner)
                        ous = half_state["ou"]

                        def recip(hx=half_ix):
                            # fold q into partitions: [128,16] costs the
                            # DVE 16 elements/lane instead of 512
                            dsb = rpool.tile([P, 16], F32, tag="den",
                                             name="den", bufs=2)
                            nc.sync.dma_start(
                                dsb[:].rearrange("p (m f) -> p m f", m=4),
                                den_dr[hx].rearrange(
                                    "m a (p f) -> (a p) m f", p=P))
                            rsb = rpool.tile([P, 16], F32, tag="rden",
                                             name="rden", bufs=2)
                            nc.vector.reciprocal(rsb, dsb)
                            nc.sync.dma_start(
                                rden_dr[hx].rearrange(
                                    "m a (p f) -> (a p) m f", p=P),
                                rsb[:].rearrange("p (m f) -> p m f", m=4))
                        items = [recip]
                        for h, t in ous:
                            items.append(
                                lambda h=h, t=t, hx=half_ix: _normalize(
                                    ot, h, t, rden_dr[hx, h % 4]))
                        if d == 1 and pair == H // 2 - 1:
                            for it in items:
                                it()
                        else:
                            deferred.extend(items)

            def _normalize(ot, h, ou_t, rden_row):
                c, po = h // 2, (h % 2) * DH
                # replicate 1/denominator across partitions with one
                # broadcast-DMA (no partition-0 staging, no gpsimd)
                rb = rpool.tile([DH, 512], F32, tag="rbc", name="rbc",
                                bufs=4)
                nc.sync.dma_start(rb[:],
                                  rden_row.to_broadcast([DH, 512]))
                mul = (nc.gpsimd.tensor_tensor if MUL_ON_G
                       else nc.vector.tensor_tensor)
                if po == 0:
                    mul(out=ot[0:DH, c, :], in0=ou_t[0:DH, :], in1=rb,
                        op=ALU.mult)
                else:
                    # engines cannot shift partitions: stage at base 0,
                    # then DMA into partitions 64-127.
                    st = rpool.tile([DH, 512], BF, tag="stg", name="stg")
                    mul(out=st, in0=ou_t[0:DH, :], in1=rb, op=ALU.mult)
                    nc.sync.dma_start(ot[po:po + DH, c, :], st)

            # ---- emission schedule -------------------------------------
            # warm the PE clock while the first DMAs land
            if HEAT_START:
                heater(HEAT_START)
            # phase A: just enough of dir-0's projections to start pair 0
            for c in range(DMC):
                emit_qt(D0, c)
            for f in range(N // 512):
                emit_kt(D0, 0, f)
            for k in range(4):
                emit_v(D0, k)

            attention(0, D0)
            attention(1, D1)
            drain_tasks()
            flush_deferred()

            # dir-1 output projection + residual + LayerNorm + store
            for q in range(QC):
                op1_final(q)

    nc.compile()
    return nc


def _prep_inputs(inputs):
    bf = ml_dtypes.bfloat16
    f = {k: np.asarray(v, dtype=np.float32) for k, v in inputs.items()}

    h2dT = np.ascontiguousarray(f["h_2d"].T).astype(bf)      # [256, 4096]
    h3dT = np.ascontiguousarray(f["h_3d"].T).astype(bf)      # [128, 4096]

    def wchunk(w, dt=bf):
        w = np.asarray(w, dtype=np.float32).astype(dt)
        return np.ascontiguousarray(w.reshape(-1, P, DM))

    bo = (f["bo23"].astype(np.float64)
          + f["bv3d"].astype(np.float64) @ f["Wo23"].astype(np.float64)
          + f["bo32"].astype(np.float64)
          + f["bv2d"].astype(np.float64) @ f["Wo32"].astype(np.float64))

    common = {
        "hkv2dT": np.ascontiguousarray(h2dT.reshape(2, P, N)),
        "hkv3dT": np.ascontiguousarray(h3dT.reshape(1, P, N)),
        "Wq2d": wchunk(f["Wq2d"]), "Wk3d": wchunk(f["Wk3d"]),
        "Wv3d": wchunk(f["Wv3d"]), "Wq3d": wchunk(f["Wq3d"]),
        "Wk2d": wchunk(f["Wk2d"]), "Wv2d": wchunk(f["Wv2d"]),
        "Wo23": wchunk(f["Wo23"]), "Wo32": wchunk(f["Wo32"]),
        "bq2dT": np.ascontiguousarray(f["bq2d"].reshape(4, P).T),
        "bq3dT": np.ascontiguousarray(f["bq3d"].reshape(4, P).T),
        "bo_sum": np.ascontiguousarray(bo.astype(np.float32)[None, :]),
        "gamma_r": np.ascontiguousarray(f["gamma"][None, :]),
        "beta_r": np.ascontiguousarray(f["beta"][None, :]),
    }

    in_maps = []
    for i in range(NCORES):
        sl = slice(i * NQ, (i + 1) * NQ)
        m = dict(common)
        m["hq2dT"] = np.ascontiguousarray(h2dT[:, sl]).reshape(2, P, NQ)
        m["hq3dT"] = np.ascontiguousarray(h3dT[:, sl]).reshape(1, P, NQ)
        in_maps.append(m)
    return in_maps


def kernel(**inputs) -> np.ndarray:
    apply_gb = not (np.allclose(np.asarray(inputs["gamma"]), 1.0)
                    and np.allclose(np.asarray(inputs["beta"]), 0.0))
    key = ("nc", apply_gb)
    if key not in _cache:
        _cache[key] = _build_program(apply_gb)
    nc = _cache[key]
    in_maps = _prep_inputs(inputs)
    res = run_bass_kernel_spmd(nc, in_maps, core_ids=list(range(NCORES)),
                               trace=TRACE)
    _cache["last_result"] = res
    return np.concatenate([r["out"] for r in res.results], axis=0)


# revision 71
# speedup vs baseline: 1.8730x; 1.0087x over previous
"""Bi-directional cross-attention kernel for Trainium2 (8 NeuronCores).

Strategy
--------
Sequence-parallel: core i owns query rows [i*512, (i+1)*512) of BOTH
attention directions and produces those rows of the final output. K/V
projections are recomputed on every core from the full h_2d/h_3d (cheaper
than an all-gather at these sizes), so no collectives are needed.

Math simplifications (done on host, exact):
  - K bias bk drops out of softmax (adds a per-query constant to every score).
  - V bias bv contributes bv @ Wo to every row (attn rows sum to 1), so it is
    folded into a single output-side bias bo_sum added before LayerNorm.
  - Q bias kept (applied during the Q^T PSUM->SBUF copy).

Device data layout (per core):
  - Everything transposed up-front on host: hT [din, n] so all matmuls are
    natural PE ops (contract over partitions).
  - Q^T, K^T stored [d_model, n] bf16.
  - V stored fp8e4m3 as [kv-chunk-pair, parity, head, 80] with a ones column
    at index 64: the fp8 DoubleRow AV matmul contracts TWO kv chunks per
    instruction and yields both O^T (rows 0-63) and the softmax denominator
    (row 64) in one accumulation.
  - Scores computed as S^T [kv, q] chunk-pairs in PSUM; the two heads of a
    pair occupy different PE row groups and stream concurrently.

Engine balance (the critical resource):
  - exp of 33.6M score elements is split between ScalarE (true ACT exp,
    fp8 out) and VectorE (Schraudolph bit-trick: p = S/ln2 + 55.67 converted
    to int8 IS the fp8e4m3 encoding of exp(S/8)).  Assignment is per-head so
    each softmax row sees one consistent approximation (bias cancels in the
    denominator).
  - scores live in four single-bank [P,512] PSUM tiles with one exp per
    tile: the scores->exp->bank-free round trip is half an iteration, so
    the PE never idles long enough for the HAM clock-gate to throttle the
    array to 1.2 GHz (the dominant cost of earlier versions).
  - PSUM->SBUF evacuations of K^T/V projections alternate Scalar/Vector.
  - softmax denominators: each head's row is parked in DRAM, a pair comes
    back as one partition-major [128,8] tile (reciprocal costs ~8
    elements/lane instead of 512), and 1/den returns via broadcast-DMAs.
  - projections are double-buffered (pj0/pj1) and spread across the whole
    attention span as deadline-paced PE filler tasks; the dir-1 output
    projection's first chunks run during the last attention pair.
"""

import math

import numpy as np
import ml_dtypes

import concourse.bass as bass
import concourse.bacc as bacc
import concourse.mybir as mybir
import concourse.tile as tile
from concourse.bass_utils import run_bass_kernel_spmd

N = 4096
D2D, D3D, DM, H, DH = 256, 128, 512, 8, 64
NCORES = 8
NQ = N // NCORES          # 512 query rows per core
EPS = 1e-5
P = 128
KC = N // P               # 32 kv chunks of 128
QC = NQ // P              # 4 query chunks of 128
DMC = DM // P             # 4 d_model chunks of 128

BF = mybir.dt.bfloat16
F8 = mybir.dt.float8e4
I8 = mybir.dt.int8
F32 = mybir.dt.float32
AF = mybir.ActivationFunctionType
ALU = mybir.AluOpType

K2 = KC // 2     # 16 kv chunk-pairs (DoubleRow contracts 2 chunks per mm)
VW = 80          # padded per-head V row width in fp8 (step % 16 == 0)

# Schraudolph exp -> fp8e4m3 bit pattern: int8(S/ln2 + B) viewed as fp8
# approximates exp(S/8).  B = 8*7 - 0.33 centers the interpolation error.
EXP_A = 1.44269504
EXP_B = 55.67

# Per-direction, per-head: True -> ScalarE ACT exp, False -> VectorE trick.
EXP_ON_S = [
    [True, False, True, False, True, False, True, False],
    [True, False, True, False, True, False, True, False],
]
EVAC_S_FRAC = 0.58  # fraction of K/V PSUM evacuations done on ScalarE
PIPELINED_EMIT = True   # scores one iteration ahead of AV
OU_ON_S = True          # even-head accumulator evac on ScalarE
MUL_ON_G = False        # gpsimd tensor_tensor thrashes IRAM libs: keep off
HEAT_START = 10         # warmup heater matmuls before phase A
HEAT_PAIR = 0           # heater matmuls at each attention pair start

TRACE = False
_cache = {}


def _build_program(apply_gb=True):
    # Bacc (not bare Bass): its compile() splits multi-semaphore waits into
    # standalone event-semaphore instructions (TRN2 allows 1 wait per inst).
    nc = bacc.Bacc("TRN2", target_bir_lowering=False)

    # ---- I/O -----------------------------------------------------------
    hq2dT = nc.dram_tensor("hq2dT", [2, P, NQ], BF, kind="ExternalInput")
    hq3dT = nc.dram_tensor("hq3dT", [1, P, NQ], BF, kind="ExternalInput")
    hkv2dT = nc.dram_tensor("hkv2dT", [2, P, N], BF, kind="ExternalInput")
    hkv3dT = nc.dram_tensor("hkv3dT", [1, P, N], BF, kind="ExternalInput")
    Wq2d = nc.dram_tensor("Wq2d", [2, P, DM], BF, kind="ExternalInput")
    Wk3d = nc.dram_tensor("Wk3d", [1, P, DM], BF, kind="ExternalInput")
    Wv3d = nc.dram_tensor("Wv3d", [1, P, DM], BF, kind="ExternalInput")
    Wq3d = nc.dram_tensor("Wq3d", [1, P, DM], BF, kind="ExternalInput")
    Wk2d = nc.dram_tensor("Wk2d", [2, P, DM], BF, kind="ExternalInput")
    Wv2d = nc.dram_tensor("Wv2d", [2, P, DM], BF, kind="ExternalInput")
    Wo23 = nc.dram_tensor("Wo23", [4, P, DM], BF, kind="ExternalInput")
    Wo32 = nc.dram_tensor("Wo32", [4, P, DM], BF, kind="ExternalInput")
    bq2dT = nc.dram_tensor("bq2dT", [P, 4], F32, kind="ExternalInput")
    bq3dT = nc.dram_tensor("bq3dT", [P, 4], F32, kind="ExternalInput")
    bo_sum = nc.dram_tensor("bo_sum", [1, DM], F32, kind="ExternalInput")
    gamma_r = nc.dram_tensor("gamma_r", [1, DM], F32, kind="ExternalInput")
    beta_r = nc.dram_tensor("beta_r", [1, DM], F32, kind="ExternalInput")
    out = nc.dram_tensor("out", [NQ, DM], F32, kind="ExternalOutput")
    # DRAM bounce buffers for the softmax denominators: SBUF-source DMAs
    # cannot broadcast or cross partitions, DRAM-source ones can.  den_dr
    # collects 4 heads' denominator rows; they come back as a [128,16]
    # partition-major tile so one reciprocal costs ~16 elements/lane.
    den_dr = nc.dram_tensor("den_dr", [8, 2, 1, 512], F32, kind="Internal")
    rden_dr = nc.dram_tensor("rden_dr", [8, 2, 1, 512], F32, kind="Internal")

    with tile.TileContext(nc) as tc:
        with (
            tc.tile_pool(name="const", bufs=1) as const,
            tc.tile_pool(name="kv", bufs=1) as kv,
            tc.tile_pool(name="epool", bufs=5) as epool,
            tc.tile_pool(name="rpool", bufs=2) as rpool,
            tc.tile_pool(name="misc", bufs=2) as misc,
            tc.tile_pool(name="psum", bufs=1, space="PSUM") as psum,
        ):
            # ---- constants / weights into SBUF -------------------------
            def load(dram, shape, dtype=BF):
                t = const.tile(shape, dtype, name=dram.name + "_sb")
                src = dram[:]
                if len(shape) == 3:
                    src = src.rearrange("o p f -> p o f")
                nc.sync.dma_start(t, src)
                return t

            # dir-0's operands first so the first projections start ASAP
            wq1 = load(Wq2d, [P, 2, DM])
            hq1 = load(hq2dT, [P, 2, NQ])
            hkv1 = load(hkv3dT, [P, 1, N])
            wk1 = load(Wk3d, [P, 1, DM])
            wv1 = load(Wv3d, [P, 1, DM])
            bq1 = const.tile([P, 4], F32, name="bq1")
            nc.sync.dma_start(bq1, bq2dT[:])
            wq2 = load(Wq3d, [P, 1, DM])
            hq2 = load(hq3dT, [P, 1, NQ])
            hkv2 = load(hkv2dT, [P, 2, N])
            wk2 = load(Wk2d, [P, 2, DM])
            wv2 = load(Wv2d, [P, 2, DM])
            bq2 = const.tile([P, 4], F32, name="bq2")
            nc.sync.dma_start(bq2, bq3dT[:])
            wo1 = load(Wo23, [P, 4, DM])
            wo2 = load(Wo32, [P, 4, DM])

            bo_bc = const.tile([P, DM], F32, name="bo_bc")
            nc.sync.dma_start(bo_bc, bo_sum[:].to_broadcast([P, DM]))
            gamma_bc = const.tile([P, DM], F32, name="gamma_bc")
            nc.sync.dma_start(gamma_bc, gamma_r[:].to_broadcast([P, DM]))
            beta_bc = const.tile([P, DM], F32, name="beta_bc")
            nc.sync.dma_start(beta_bc, beta_r[:].to_broadcast([P, DM]))
            eps_sb = const.tile([P, 1], F32, name="eps_sb")
            nc.vector.memset(eps_sb, EPS)

            # HAM heater: dummy matmuls keep the PE activity monitor from
            # throttling the array clock to 1.2 GHz during engine-bound
            # stretches. Reads a memset tile, writes an unread pj tile.
            heat = const.tile([P, 256], BF, name="heat")
            nc.vector.memset(heat, 0.0)

            def heater(n):
                for _ in range(n):
                    nc.tensor.matmul(pj()[:, 0:256], lhsT=heat[:, 0:P],
                                     rhs=heat[:], start=True, stop=True)

            x1 = kv.tile([P, QC, DM], F32, name="x1")
            kt1 = kv.tile([P, DMC, N], BF, name="kt1")
            vv1 = kv.tile([P, K2, 2, H, VW], F8, name="vv1")
            qt1 = kv.tile([P, DMC, NQ], BF, name="qt1")
            ot1 = kv.tile([P, DMC, NQ], BF, name="ot1")
            kt2 = kv.tile([P, DMC, N], BF, name="kt2")
            vv2 = kv.tile([P, K2, 2, H, VW], F8, name="vv2")
            qt2 = kv.tile([P, DMC, NQ], BF, name="qt2")
            ot2 = kv.tile([P, DMC, NQ], BF, name="ot2")

            # the fp8 ones-columns used by the DoubleRow denominator rows,
            # written once (disjoint from the per-chunk V data columns)
            nc.vector.memset(vv1[:, :, :, :, DH:DH + 1], 1.0)
            nc.vector.memset(vv2[:, :, :, :, DH:DH + 1], 1.0)

            dirs = [
                dict(hq=hq1, nq=2, hkv=hkv1, nk=1, wq=wq1, wk=wk1, wv=wv1,
                     wo=wo1, bq=bq1, kt=kt1, vv=vv1, qt=qt1, ot=ot1),
                dict(hq=hq2, nq=1, hkv=hkv2, nk=2, wq=wq2, wk=wk2, wv=wv2,
                     wo=wo2, bq=bq2, kt=kt2, vv=vv2, qt=qt2, ot=ot2),
            ]

            # ---- projection emitters (double-buffered pj0/pj1 PSUM) ----
            pj_i = [0]

            def pj():
                t = psum.tile([P, 512], F32, tag=f"pj{pj_i[0] % 2}",
                              bufs=1, name="pj")
                pj_i[0] += 1
                return t

            ev_acc = [0.0]

            def evac_on_s():
                ev_acc[0] += EVAC_S_FRAC
                if ev_acc[0] >= 1.0 - 1e-9:
                    ev_acc[0] -= 1.0
                    return True
                return False

            def emit_qt(D, c):
                ps = pj()
                for dc in range(D["nq"]):
                    nc.tensor.matmul(
                        ps, lhsT=D["wq"][:, dc, c * P:(c + 1) * P],
                        rhs=D["hq"][:, dc, :],
                        start=(dc == 0), stop=(dc == D["nq"] - 1))
                # bias-add rides the PSUM->SBUF copy on ScalarE
                nc.scalar.activation(
                    out=D["qt"][:, c, :], in_=ps, func=AF.Identity,
                    bias=D["bq"][:, c:c + 1])

            def emit_kt(D, c, f):
                ps = pj()
                for dc in range(D["nk"]):
                    nc.tensor.matmul(
                        ps, lhsT=D["wk"][:, dc, c * P:(c + 1) * P],
                        rhs=D["hkv"][:, dc, f * 512:(f + 1) * 512],
                        start=(dc == 0), stop=(dc == D["nk"] - 1))
                dst = D["kt"][:, c, f * 512:(f + 1) * 512]
                if evac_on_s():
                    nc.scalar.activation(out=dst, in_=ps, func=AF.Identity)
                else:
                    nc.vector.tensor_copy(out=dst, in_=ps)

            def emit_v(D, k):
                ps = pj()
                for dc in range(D["nk"]):
                    nc.tensor.matmul(
                        ps, lhsT=D["hkv"][:, dc, k * P:(k + 1) * P],
                        rhs=D["wv"][:, dc, :],
                        start=(dc == 0), stop=(dc == D["nk"] - 1))
                dst = D["vv"][:, k // 2, k % 2, :, 0:DH]
                src = ps.rearrange("p (h e) -> p h e", h=H)
                if evac_on_s():
                    nc.scalar.activation(out=dst, in_=src, func=AF.Identity)
                else:
                    nc.vector.tensor_copy(out=dst, in_=src)

            def outproj_mms(D, q, ps, mc0, mc1, start):
                for mc in range(mc0, mc1):
                    nc.tensor.matmul(
                        ps, lhsT=D["ot"][:, mc, q * P:(q + 1) * P],
                        rhs=D["wo"][:, mc, :],
                        start=(start and mc == mc0),
                        stop=(mc == DMC - 1))

            def emit_outproj(D, q, d):
                ps = pj()
                outproj_mms(D, q, ps, 0, DMC, True)
                finish_outproj(ps, q, d)

            # outproj1 rows q=0,1 run their first 3 contraction chunks
            # during dir-1's last pair (inputs already normalized), so the
            # tail only owes them one matmul + LayerNorm each
            op1_ps = {}

            def op1_partial(q, tag=None):
                if tag is None:
                    ps = pj()
                else:
                    # after the last scores the u banks are free
                    ps = psum.tile([P, 512], F32, tag=tag, name="op1")
                op1_ps[q] = ps
                outproj_mms(dirs[1], q, ps, 0, DMC - 1, True)

            def op1_final(q):
                if q in op1_ps:
                    ps = op1_ps[q]
                    outproj_mms(dirs[1], q, ps, DMC - 1, DMC, False)
                else:
                    ps = pj()
                    outproj_mms(dirs[1], q, ps, 0, DMC, True)
                finish_outproj(ps, q, 1)

            def finish_outproj(ps, q, d):
                if d == 0:
                    # x1 = o23 + bo_sum
                    nc.vector.tensor_add(out=x1[:, q, :], in0=ps, in1=bo_bc)
                else:
                    # x = o32 + x1 ; LayerNorm ; write out
                    x_t = misc.tile([P, DM], F32, tag="x_t", name="x_t")
                    nc.vector.tensor_add(out=x_t, in0=ps, in1=x1[:, q, :])
                    stats = misc.tile([P, 6], F32, tag="stats", name="stats")
                    nc.vector.bn_stats(out=stats, in_=x_t)
                    mv = misc.tile([P, 2], F32, tag="mv", name="mv")
                    nc.vector.bn_aggr(out=mv, in_=stats)
                    std = misc.tile([P, 1], F32, tag="std", name="std")
                    nc.scalar.activation(out=std, in_=mv[:, 1:2],
                                         func=AF.Sqrt, bias=eps_sb[:, 0:1])
                    rstd = misc.tile([P, 1], F32, tag="rstd", name="rstd")
                    nc.vector.reciprocal(rstd, std)
                    nc.vector.tensor_scalar(
                        out=x_t, in0=x_t, scalar1=mv[:, 0:1],
                        scalar2=rstd, op0=ALU.subtract, op1=ALU.mult)
                    if apply_gb:
                        nc.vector.tensor_mul(out=x_t, in0=x_t,
                                             in1=gamma_bc)
                        nc.vector.tensor_add(out=x_t, in0=x_t,
                                             in1=beta_bc)
                    nc.sync.dma_start(out[q * P:(q + 1) * P, :], x_t)

            # ---- deadline-paced filler tasks ---------------------------
            # global attention iterations: dir-0 -> 0..63, dir-1 -> 64..127
            D0, D1 = dirs
            tasks = []
            for k in range(4, KC):
                tasks.append((max(0, k // 2 - 1), lambda k=k: emit_v(D0, k)))
            for c in range(1, DMC):
                for f in range(N // 512):
                    tasks.append((16 * c - 2,
                                  lambda c=c, f=f: emit_kt(D0, c, f)))
            for c in range(DMC):
                tasks.append((60 + 16 * c, lambda c=c: emit_qt(D1, c)))
            for c in range(DMC):
                for f in range(N // 512):
                    tasks.append((62 + 16 * c,
                                  lambda c=c, f=f: emit_kt(D1, c, f)))
            for k in range(KC):
                tasks.append((62 + k // 2, lambda k=k: emit_v(D1, k)))
            for q in range(QC):
                tasks.append((74 + 12 * q,
                              lambda q=q: emit_outproj(D0, q, 0)))
            tasks.sort(key=lambda x: x[0])
            n_tasks = len(tasks)
            fstate = {"done": 0, "it": 0}

            def filler():
                t = fstate["it"]
                fstate["it"] += 1
                uniform = math.ceil(n_tasks * (t + 1) / 122.0)
                while fstate["done"] < n_tasks and (
                        tasks[fstate["done"]][0] <= t
                        or fstate["done"] < uniform):
                    tasks[fstate["done"]][1]()
                    fstate["done"] += 1

            def drain_tasks():
                while fstate["done"] < n_tasks:
                    tasks[fstate["done"]][1]()
                    fstate["done"] += 1

            # ---- attention ---------------------------------------------
            deferred = []

            def pop_deferred():
                if deferred:
                    deferred.pop(0)()

            def flush_deferred():
                while deferred:
                    deferred.pop(0)()

            def attention(d, D):
                kt, vv, qt, ot = D["kt"], D["vv"], D["qt"], D["ot"]
                srot = [0]

                def emit_scores_exp(pair, k2):
                    # [P,512] score tiles + per-half exp: the bank a score
                    # matmul writes is freed by one 512-wide exp, halving
                    # the scores->exp->reuse round trip that stalls the PE
                    heads = (2 * pair, 2 * pair + 1)
                    e = {}
                    for h in heads:
                        e[h] = epool.tile([P, 2, 512], F8, tag="E",
                                          name="e_t")
                    for j in range(2):
                        k = 2 * k2 + j
                        s = {}
                        for h in heads:
                            s[h] = psum.tile([P, 512], F32,
                                             tag=f"u{srot[0] % 4}",
                                             name="s_u")
                            srot[0] += 1
                        # the two heads sit in different PE row groups and
                        # stream concurrently
                        for h in heads:
                            c, po = h // 2, (h % 2) * DH
                            nc.tensor.matmul(
                                s[h],
                                lhsT=kt[po:po + DH, c, k * P:(k + 1) * P],
                                rhs=qt[po:po + DH, c, :],
                                start=True, stop=True)
                        for h in heads:
                            dst = e[h][:, j, :]
                            if EXP_ON_S[d][h]:
                                nc.scalar.activation(out=dst, in_=s[h],
                                                     func=AF.Exp,
                                                     scale=0.125)
                            else:
                                nc.vector.tensor_scalar(
                                    out=dst.bitcast(I8), in0=s[h],
                                    scalar1=EXP_A, scalar2=EXP_B,
                                    op0=ALU.mult, op1=ALU.add)
                    return e

                for pair in range(H // 2):
                    if d == 1 and pair == H // 2 - 1:
                        deferred.append(lambda: op1_partial(0))
                        deferred.append(lambda: op1_partial(1))
                    heads = (2 * pair, 2 * pair + 1)
                    o_ps = {h: psum.tile([P, 512], F32, tag=f"o{h % 2}",
                                         name=f"o_ps{h % 2}")
                            for h in heads}
                    def emit_av(k2, e):
                        for h in heads:
                            # fp8 DoubleRow AV: contracts both kv chunks in
                            # one matmul; vv col 64 is ones so row 64
                            # accumulates the softmax denominator
                            nc.tensor.matmul(
                                o_ps[h][0:DH + 1, :],
                                lhsT=vv[:, k2, :, h, 0:DH + 1],
                                rhs=e[h],
                                start=(k2 == 0), stop=(k2 == K2 - 1),
                                perf_mode=mybir.MatmulPerfMode.DoubleRow,
                                skip_group_check=True)

                    if PIPELINED_EMIT:
                        e_cur = emit_scores_exp(pair, 0)
                        if HEAT_PAIR:
                            heater(HEAT_PAIR)
                        for k2 in range(K2):
                            if k2 + 1 < K2:
                                e_nxt = emit_scores_exp(pair, k2 + 1)
                            # previous pairs' normalization work, one piece
                            # per iteration (their o banks were already
                            # freed by the ou evacuations)
                            pop_deferred()
                            filler()
                            emit_av(k2, e_cur)
                            if k2 + 1 < K2:
                                e_cur = e_nxt
                    else:
                        for k2 in range(K2):
                            e = emit_scores_exp(pair, k2)
                            pop_deferred()
                            emit_av(k2, e)
                            filler()
                    if d == 1 and pair == H // 2 - 1:
                        # fill the final normalize's DMA-latency gap with
                        # the rest of the output projection's partials
                        op1_partial(2, "u0")
                        op1_partial(3, "u1")
                    # pair end: evacuate both accumulators fast (frees the
                    # o banks before the next pair's first AV) ...
                    slot = d * 4 + pair
                    ous = []
                    for h in heads:
                        t = rpool.tile([DH + 1, 512], F32, tag="ou",
                                       name="ou", bufs=4)
                        if h % 2 == 0 and OU_ON_S:
                            nc.scalar.activation(out=t,
                                                 in_=o_ps[h][0:DH + 1, :],
                                                 func=AF.Identity)
                        else:
                            nc.vector.tensor_copy(out=t,
                                                  in_=o_ps[h][0:DH + 1, :])
                        # park this head's denominator row in DRAM; one
                        # reciprocal serves the pair
                        nc.sync.dma_start(den_dr[slot, h % 2],
                                          t[DH:DH + 1, :])
                        ous.append((h, t))
                    # ... then normalize the pair lazily, one piece per
                    # upcoming iteration (eagerly for the very last pair
                    # so the output projection can start sooner)

                    def recip(sx=slot):
                        # fold q into partitions: [128,8] costs the DVE
                        # 8 elements/lane instead of 512
                        dsb = rpool.tile([P, 8], F32, tag="den",
                                         name="den", bufs=2)
                        nc.sync.dma_start(
                            dsb[:].rearrange("p (m f) -> p m f", m=2),
                            den_dr[sx].rearrange(
                                "m a (p f) -> (a p) m f", p=P))
                        rsb = rpool.tile([P, 8], F32, tag="rden",
                                         name="rden", bufs=2)
                        nc.vector.reciprocal(rsb, dsb)
                        nc.sync.dma_start(
                            rden_dr[sx].rearrange(
                                "m a (p f) -> (a p) m f", p=P),
                            rsb[:].rearrange("p (m f) -> p m f", m=2))
                    items = [recip]
                    for h, t in ous:
                        items.append(
                            lambda h=h, t=t, sx=slot: _normalize(
                                ot, h, t, rden_dr[sx, h % 2]))
                    if d == 1 and pair == H // 2 - 1:
                        for it in items:
                            it()
                    else:
                        deferred.extend(items)

            def _normalize(ot, h, ou_t, rden_row):
                c, po = h // 2, (h % 2) * DH
                # replicate 1/denominator across partitions with one
                # broadcast-DMA (no partition-0 staging, no gpsimd)
                rb = rpool.tile([DH, 512], F32, tag="rbc", name="rbc",
                                bufs=4)
                nc.sync.dma_start(rb[:],
                                  rden_row.to_broadcast([DH, 512]))
                mul = (nc.gpsimd.tensor_tensor if MUL_ON_G
                       else nc.vector.tensor_tensor)
                if po == 0:
                    mul(out=ot[0:DH, c, :], in0=ou_t[0:DH, :], in1=rb,
                        op=ALU.mult)
                else:
                    # engines cannot shift partitions: stage at base 0,
                    # then DMA into partitions 64-127.
                    st = rpool.tile([DH, 512], BF, tag="stg", name="stg")
                    mul(out=st, in0=ou_t[0:DH, :], in1=rb, op=ALU.mult)
                    nc.sync.dma_start(ot[po:po + DH, c, :], st)

            # ---- emission schedule -------------------------------------
            # warm the PE clock while the first DMAs land
            if HEAT_START:
                heater(HEAT_START)
            # phase A: just enough of dir-0's projections to start pair 0
            for c in range(DMC):
                emit_qt(D0, c)
            for f in range(N // 512):
                emit_kt(D0, 0, f)
            for k in range(4):
                emit_v(D0, k)

            attention(0, D0)
            attention(1, D1)
            drain_tasks()
            flush_deferred()

            # dir-1 output projection + residual + LayerNorm + store
            for q in range(QC):
                op1_final(q)

    nc.compile()
    return nc


def _prep_inputs(inputs):
    bf = ml_dtypes.bfloat16
    f = {k: np.asarray(v, dtype=np.float32) for k, v in inputs.items()}

    h2dT = np.ascontiguousarray(f["h_2d"].T).astype(bf)      # [256, 4096]
    h3dT = np.ascontiguousarray(f["h_3d"].T).astype(bf)      # [128, 4096]

    def wchunk(w, dt=bf):
        w = np.asarray(w, dtype=np.float32).astype(dt)
        return np.ascontiguousarray(w.reshape(-1, P, DM))

    bo = (f["bo23"].astype(np.float64)
          + f["bv3d"].astype(np.float64) @ f["Wo23"].astype(np.float64)
          + f["bo32"].astype(np.float64)
          + f["bv2d"].astype(np.float64) @ f["Wo32"].astype(np.float64))

    common = {
        "hkv2dT": np.ascontiguousarray(h2dT.reshape(2, P, N)),
        "hkv3dT": np.ascontiguousarray(h3dT.reshape(1, P, N)),
        "Wq2d": wchunk(f["Wq2d"]), "Wk3d": wchunk(f["Wk3d"]),
        "Wv3d": wchunk(f["Wv3d"]), "Wq3d": wchunk(f["Wq3d"]),
        "Wk2d": wchunk(f["Wk2d"]), "Wv2d": wchunk(f["Wv2d"]),
        "Wo23": wchunk(f["Wo23"]), "Wo32": wchunk(f["Wo32"]),
        "bq2dT": np.ascontiguousarray(f["bq2d"].reshape(4, P).T),
        "bq3dT": np.ascontiguousarray(f["bq3d"].reshape(4, P).T),
        "bo_sum": np.ascontiguousarray(bo.astype(np.float32)[None, :]),
        "gamma_r": np.ascontiguousarray(f["gamma"][None, :]),
        "beta_r": np.ascontiguousarray(f["beta"][None, :]),
    }

    in_maps = []
    for i in range(NCORES):
        sl = slice(i * NQ, (i + 1) * NQ)
        m = dict(common)
        m["hq2dT"] = np.ascontiguousarray(h2dT[:, sl]).reshape(2, P, NQ)
        m["hq3dT"] = np.ascontiguousarray(h3dT[:, sl]).reshape(1, P, NQ)
        in_maps.append(m)
    return in_maps


def kernel(**inputs) -> np.ndarray:
    apply_gb = not (np.allclose(np.asarray(inputs["gamma"]), 1.0)
                    and np.allclose(np.asarray(inputs["beta"]), 0.0))
    key = ("nc", apply_gb)
    if key not in _cache:
        _cache[key] = _build_program(apply_gb)
    nc = _cache[key]
    in_maps = _prep_inputs(inputs)
    res = run_bass_kernel_spmd(nc, in_maps, core_ids=list(range(NCORES)),
                               trace=TRACE)
    _cache["last_result"] = res
    return np.concatenate([r["out"] for r in res.results], axis=0)
